# revision 10
# baseline (speedup 1.0000x reference)
"""Bass/Tile attention kernel for TRN2 — per-core program builder.

Sharding (v3, tensor-parallel over heads): core c of 8 handles batch
b = c//2 and head-half h = c%2 (8 of 16 heads). Each core projects
Q/K/V for its 8 heads over ALL 2048 tokens (column-shard of Wq/Wk/Wv),
runs attention, and computes a PARTIAL output projection against its
row-shard of Wo. The host sums the two partials per batch and adds bo.
vs. the query-sharded v2 this halves the K/V projection matmul work
(no duplicate K/V proj per query half).

Per-core DRAM inputs:
  xt  : [D, S]   bf16  X[b]^T (full)
  wq/wk/wv : [D, DQ=512] bf16 column shards; wo : [DQ, D] bf16 row shard
  bq/bk : [DQ] fp32; bv : [DQ] bf16
Output:
  ot  : [D, S] fp32  partial O^T (host: out = (ot0+ot1)^T + bo)

Schedule: the kc loop is software-pipelined and emitted in the order
[ACT(kc+1), scores(kc+2), filler, PV(kc)] so that (engines execute
their queues in order) the PE never head-blocks while ScalarE's exp
runs, and projection-chain slices (2 matmuls each, deadline-ordered)
fill the remaining PE slack. PSUM (8 banks): scores 2x[128,2,512]
double-buffered (4), z chains za/zb (2, evicted to SBUF right after
each stream), projection chains double-buffered (2).

Attention per (pair of heads, 512-query chunk), per kc (128 keys):
  scores^T[k,q] both heads -> PSUM [128,2,512] (row-packed matmuls,
  head-lo rows 0-63 / head-hi rows 64-127); probs = exp(0.125*scores)
  as one ACT [128,1024] -> bf16; PV per head with augmented [V_h | 1]
  stationary (M=65) accumulating z + rowsum over kc. Normalization
  divides by the rowsum via a DRAM-bounce partition-broadcast of the
  reciprocal (SBUF->SBUF broadcast DMA is illegal; the custom recip op
  needs fp32 SBUF at base partition 0).
"""

import numpy as np
import ml_dtypes
from contextlib import ExitStack

import concourse.bass as bass
import concourse.tile as tile
from concourse import bacc, mybir

F32 = mybir.dt.float32
BF16 = mybir.dt.bfloat16
P = 128


def build_attention_nc(S_full=2048, D=1024, DQ=512):
    NPAIR = DQ // P           # head pairs (128 dims each) = 4
    KC = S_full // P          # k chunks = 16
    TOKC = S_full // 512      # 512-token chunks (K proj / queries) = 4
    NTOK = S_full // P        # 128-token chunks for V proj = 16
    QCN = S_full // 512       # 512-query chunks = 4
    DIN = D // P              # input-dim chunks = 8
    DCO = DQ // P             # o-proj contraction chunks = 4

    nc = bacc.Bacc("TRN2", target_bir_lowering=False, debug=False)

    xt_d = nc.dram_tensor("xt", [D, S_full], BF16, kind="ExternalInput").ap()
    wq_d = nc.dram_tensor("wq", [D, DQ], BF16, kind="ExternalInput").ap()
    wk_d = nc.dram_tensor("wk", [D, DQ], BF16, kind="ExternalInput").ap()
    wv_d = nc.dram_tensor("wv", [D, DQ], BF16, kind="ExternalInput").ap()
    wo_d = nc.dram_tensor("wo", [DQ, D], BF16, kind="ExternalInput").ap()
    bq_d = nc.dram_tensor("bq", [DQ], F32, kind="ExternalInput").ap()
    bk_d = nc.dram_tensor("bk", [DQ], F32, kind="ExternalInput").ap()
    bv_d = nc.dram_tensor("bv", [DQ], BF16, kind="ExternalInput").ap()
    ot_d = nc.dram_tensor("ot", [D, S_full], F32, kind="ExternalOutput").ap()

    xt_r = xt_d.rearrange("(c p) t -> p c t", p=P)
    wq_r = wq_d.rearrange("(c p) n -> p c n", p=P)
    wk_r = wk_d.rearrange("(c p) n -> p c n", p=P)
    wv_r = wv_d.rearrange("(c p) n -> p c n", p=P)
    wo_r = wo_d.rearrange("(c p) n -> p c n", p=P)
    bq_r = bq_d.rearrange("(c p) -> p c", p=P)
    bk_r = bk_d.rearrange("(c p) -> p c", p=P)
    bv_r = bv_d.rearrange("(a d) -> a d", a=1)

    EXP = mybir.ActivationFunctionType.Exp

    with tile.TileContext(nc) as tc, ExitStack() as ctx:
        const = ctx.enter_context(tc.tile_pool(name="const", bufs=1))
        big = ctx.enter_context(tc.tile_pool(name="big", bufs=1))
        wpool = ctx.enter_context(tc.tile_pool(name="wpool", bufs=4))
        work = ctx.enter_context(tc.tile_pool(name="work", bufs=3))
        probs_pool = ctx.enter_context(tc.tile_pool(name="probs", bufs=3))
        zsb = ctx.enter_context(tc.tile_pool(name="zsb", bufs=4))
        # PSUM budget (8 banks): scores 2x2 + z 2 + proj 2
        spsum = ctx.enter_context(tc.tile_pool(name="spsum", bufs=2, space="PSUM"))
        zpsum = ctx.enter_context(tc.tile_pool(name="zpsum", bufs=2, space="PSUM"))
        ppsum = ctx.enter_context(tc.tile_pool(name="ppsum", bufs=2, space="PSUM"))
        dramp = ctx.enter_context(tc.tile_pool(name="dramp", bufs=2, space="DRAM"))

        # ---- constants ----
        bq_t = const.tile([P, NPAIR], F32)
        nc.sync.dma_start(bq_t[:], bq_r[:, :])
        bk_t = const.tile([P, NPAIR], F32)
        nc.sync.dma_start(bk_t[:], bk_r[:, :])
        bvb = const.tile([P, DQ], BF16)
        nc.sync.dma_start(bvb[:], bv_r[0:1, :].to_broadcast((P, DQ)))

        # ---- X^T + weight loads. Ordered so the first attention stream
        # can start ~12us in: wk + first 512-token block of xt, then wq
        # (first scores), then wv (first PV), then the remaining xt
        # blocks (consumed progressively by K/V chains), wo last.
        xt_t = big.tile([P, DIN, S_full], BF16, tag="xt")
        wk_t = wpool.tile([P, DIN, DQ], BF16, tag="w", name="wk_t")
        wq_t = wpool.tile([P, DIN, DQ], BF16, tag="w", name="wq_t")
        wv_t = wpool.tile([P, DIN, DQ], BF16, tag="w", name="wv_t")
        wo_t = wpool.tile([P, DCO, D], BF16, tag="wo", name="wo_t")
        for c in range(DIN):
            nc.sync.dma_start(wk_t[:, c, :], wk_r[:, c, :])
            nc.sync.dma_start(xt_t[:, c, 0:512], xt_r[:, c, 0:512])
        for c in range(DIN):
            nc.sync.dma_start(wq_t[:, c, :], wq_r[:, c, :])
        for c in range(DIN):
            nc.sync.dma_start(wv_t[:, c, :], wv_r[:, c, :])
        for tb in range(1, TOKC):
            for c in range(DIN):
                nc.sync.dma_start(xt_t[:, c, tb * 512:(tb + 1) * 512],
                                  xt_r[:, c, tb * 512:(tb + 1) * 512])
        for c in range(DCO):
            nc.sync.dma_start(wo_t[:, c, :], wo_r[:, c, :])

        kt = [big.tile([P, S_full], BF16, tag=f"kt{p}", name=f"kt{p}")
              for p in range(NPAIR)]
        qt = [big.tile([P, S_full], BF16, tag=f"qt{p}", name=f"qt{p}")
              for p in range(NPAIR)]
        # per-pair augmented V: 2 heads x (64 data + 1 ones col)
        vt = [big.tile([P, NTOK, 130], BF16, tag=f"vt{p}", name=f"vt{p}")
              for p in range(NPAIR)]
        zt = [big.tile([P, S_full], BF16, tag=f"zt{p}", name=f"zt{p}")
              for p in range(NPAIR)]

        for pr in range(NPAIR):
            ones_ap = vt[pr].rearrange("p t (h c) -> p t h c", c=65)[:, :, :, 64:65]
            nc.vector.memset(ones_ap, 1.0)

        # ---- projection chain thunks (each thunk emits ~2 matmuls) ----
        def k_chain(pr, t):
            st = {}
            def mk(c0):
                def f():
                    if c0 == 0:
                        st['ps'] = ppsum.tile([P, 512], F32, tag="proj",
                                              name="psk")
                    for c in (c0, c0 + 1):
                        nc.tensor.matmul(
                            st['ps'][:],
                            wk_t[:, c, pr * P:(pr + 1) * P],
                            xt_t[:, c, t * 512:(t + 1) * 512],
                            start=(c == 0), stop=(c == DIN - 1),
                        )
                    if c0 == DIN - 2:
                        nc.vector.tensor_scalar_add(
                            kt[pr][:, t * 512:(t + 1) * 512], st['ps'][:],
                            bk_t[:, pr:pr + 1])
                return f
            return [mk(c0) for c0 in range(0, DIN, 2)]

        def q_chain(pr, t):
            st = {}
            def mk(c0):
                def f():
                    if c0 == 0:
                        st['ps'] = ppsum.tile([P, 512], F32, tag="proj",
                                              name="psq")
                    for c in (c0, c0 + 1):
                        nc.tensor.matmul(
                            st['ps'][:],
                            wq_t[:, c, pr * P:(pr + 1) * P],
                            xt_t[:, c, t * 512:(t + 1) * 512],
                            start=(c == 0), stop=(c == DIN - 1),
                        )
                    if c0 == DIN - 2:
                        nc.vector.tensor_scalar_add(
                            qt[pr][:, t * 512:(t + 1) * 512], st['ps'][:],
                            bq_t[:, pr:pr + 1])
                return f
            return [mk(c0) for c0 in range(0, DIN, 2)]

        def v_chain(g, t):
            prs = (2 * g, 2 * g + 1)
            st = {}
            def mk(c0):
                def f():
                    if c0 == 0:
                        st['ps'] = ppsum.tile([P, 256], F32, tag="proj",
                                              name="psv")
                    for c in (c0, c0 + 1):
                        nc.tensor.matmul(
                            st['ps'][:],
                            xt_t[:, c, t * P:(t + 1) * P],
                            wv_t[:, c, g * 256:(g + 1) * 256],
                            start=(c == 0), stop=(c == DIN - 1),
                        )
                    if c0 == DIN - 2:
                        for u in range(2):
                            dst = vt[prs[u]].rearrange(
                                "p t (h c) -> p t h c", c=65)[:, t, :, 0:64]
                            src = st['ps'][:, u * 128:(u + 1) * 128].rearrange(
                                "p (h c) -> p h c", c=64)
                            bsrc = bvb[:, (g * 256 + u * 128):
                                       (g * 256 + (u + 1) * 128)
                                       ].rearrange("p (h c) -> p h c", c=64)
                            nc.vector.tensor_add(dst, src, bsrc)
                return f
            return [mk(c0) for c0 in range(0, DIN, 2)]

        def o_chain(qc, ec):
            oqsl = slice(qc * 512, (qc + 1) * 512)
            st = {}
            def mk(d0):
                def f():
                    if d0 == 0:
                        st['ps'] = ppsum.tile([P, 512], F32, tag="proj",
                                              name="pso")
                    for dc in (d0, d0 + 1):
                        nc.tensor.matmul(
                            st['ps'][:],
                            wo_t[:, dc, ec * P:(ec + 1) * P],
                            zt[dc][:, oqsl],
                            start=(dc == 0), stop=(dc == DCO - 1),
                        )
                    if d0 == DCO - 2:
                        stg = work.tile([P, 512], F32, tag="stage",
                                        name="st", bufs=2)
                        nc.vector.tensor_copy(stg[:], st['ps'][:])
                        nc.sync.dma_start(
                            ot_d[ec * P:(ec + 1) * P, oqsl], stg[:])
                return f
            return [mk(d0) for d0 in range(0, DCO, 2)]

        # ---- filler queue: (deadline_stream, thunk), emitted in order ----
        fillers = []

        def push(deadline, thunks):
            for th in thunks:
                fillers.append((deadline, th))

        # head work (before stream 0): K(0,t0), Q(0,qc0), V(quad0) t0/t1;
        # the rest of K(0)/V(quad0) is emitted inside stream 0 (V chains
        # just-in-time, one per kc, two iterations ahead of the PV that
        # consumes them).
        for th in k_chain(0, 0):
            th()
        for th in q_chain(0, 0):
            th()
        for th in v_chain(0, 0):
            th()
        for th in v_chain(0, 1):
            th()

        # stream s = pr*4 + qc; deadline = stream during which to emit.
        # Every producer's deadline is < its first consumer stream; loads
        # are spread so budget-2 pops drain each stream's quota in-stream.
        for t in range(1, TOKC):
            push(0, k_chain(0, t))
        push(0, q_chain(0, 1))
        push(1, k_chain(1, 0)); push(1, k_chain(1, 1))
        push(1, q_chain(0, 2))
        push(2, k_chain(1, 2)); push(2, k_chain(1, 3))
        push(2, q_chain(0, 3))
        push(3, q_chain(1, 0))
        for t in range(NTOK):
            push(4 + t // 4, v_chain(1, t))
        push(4, q_chain(1, 1))
        push(5, q_chain(1, 2))
        push(6, k_chain(2, 0)); push(6, k_chain(2, 1))
        push(6, q_chain(1, 3))
        push(7, k_chain(2, 2)); push(7, k_chain(2, 3))
        push(7, q_chain(2, 0))
        push(8, q_chain(2, 1))
        push(9, q_chain(2, 2))
        push(10, k_chain(3, 0)); push(10, k_chain(3, 1))
        push(10, q_chain(2, 3))
        push(11, k_chain(3, 2)); push(11, k_chain(3, 3))
        push(11, q_chain(3, 0))
        push(12, q_chain(3, 1))
        push(13, [th for ec in range(DIN) for th in o_chain(0, ec)])
        push(13, q_chain(3, 2))
        push(14, [th for ec in range(DIN) for th in o_chain(1, ec)])
        push(14, q_chain(3, 3))
        push(15, [th for ec in range(DIN) for th in o_chain(2, ec)])
        push(16, [th for ec in range(DIN) for th in o_chain(3, ec)])

        fillers.sort(key=lambda x: x[0])
        fillers = list(reversed(fillers))  # pop from end

        def pop_fillers(s, budget):
            n = 0
            while fillers and (fillers[-1][0] < s
                               or (fillers[-1][0] <= s and n < budget)):
                fillers.pop()[1]()
                n += 1

        # ---- one attention stream: (pair pr, 512-query chunk qc) ----
        def attn(pr, qc, s):
            # Emission order IS program semantics (Tile serializes a
            # later-emitted write AFTER an earlier-emitted read of the same
            # region), so every producer chain with deadline < s must be
            # fully emitted before this stream's first scores read.
            pop_fillers(s, 0)
            vpr = vt[pr].rearrange("p t (h c) -> p t h c", c=65)
            qsl = slice(qc * 512, (qc + 1) * 512)
            za = zpsum.tile([P, 512], F32, tag="z", name=f"za{pr}_{qc}")
            zb = zpsum.tile([P, 512], F32, tag="z", name=f"zb{pr}_{qc}")

            def emit_scores(kc):
                sq = spsum.tile([P, 2, 512], F32, tag="sc", name="sq")
                nc.tensor.matmul(
                    sq[:, 0, :],
                    kt[pr][0:64, kc * P:(kc + 1) * P],
                    qt[pr][0:64, qsl],
                    start=True, stop=True,
                )
                nc.tensor.matmul(
                    sq[:, 1, :],
                    kt[pr][64:128, kc * P:(kc + 1) * P],
                    qt[pr][64:128, qsl],
                    start=True, stop=True,
                )
                return sq

            def emit_act(sq):
                pq = probs_pool.tile([P, 2, 512], BF16, tag="probs",
                                     name="pq")
                nc.scalar.activation(pq[:], sq[:], EXP, scale=0.125)
                return pq

            def emit_pv(kc, pq):
                nc.tensor.matmul(
                    za[0:65, :], vpr[:, kc, 0, :], pq[:, 0, :],
                    start=(kc == 0), stop=(kc == KC - 1),
                )
                nc.tensor.matmul(
                    zb[0:65, :], vpr[:, kc, 1, :], pq[:, 1, :],
                    start=(kc == 0), stop=(kc == KC - 1),
                )

            # software pipeline: ACT(kc+1), scores(kc+2), fillers, PV(kc)
            sq0 = emit_scores(0)
            pq_cur = emit_act(sq0)
            sq_next = emit_scores(1)
            for kc in range(KC):
                pq_next = None
                if kc + 1 < KC:
                    pq_next = emit_act(sq_next)
                if kc + 2 < KC:
                    sq_next = emit_scores(kc + 2)
                pop_fillers(s, 2)
                emit_pv(kc, pq_cur)
                if s == 0 and kc + 2 < KC:
                    # just-in-time V(quad0) chain for kc+2, two iterations
                    # ahead of the PV that reads it
                    for th in v_chain(0, kc + 2):
                        th()
                pq_cur = pq_next

            # evict z chains to SBUF (frees the z banks), then normalize
            zaf = zsb.tile([P, 512], F32, tag="zf", name="zaf")
            nc.vector.tensor_copy(zaf[0:65, :], za[0:65, :])
            zbf = zsb.tile([P, 512], F32, tag="zf", name="zbf")
            nc.vector.tensor_copy(zbf[0:65, :], zb[0:65, :])
            rsc = dramp.tile([2, 512], F32, tag="rsc", name="rsc")
            nc.sync.dma_start(rsc[0:1, :], zaf[64:65, :])
            nc.sync.dma_start(rsc[1:2, :], zbf[64:65, :])
            rbr = work.tile([P, 2, 512], F32, tag="rbc", name="rbr", bufs=1)
            nc.sync.dma_start(rbr[0:64, 0, :],
                              rsc[0:1, :].to_broadcast((64, 512)))
            nc.sync.dma_start(rbr[0:64, 1, :],
                              rsc[1:2, :].to_broadcast((64, 512)))
            rb = work.tile([P, 2, 512], F32, tag="rbc2", name="rb", bufs=1)
            nc.vector.reciprocal_approx_fast(rb[0:64, :, :],
                                             rbr[0:64, :, :])
            nc.vector.tensor_mul(zt[pr][0:64, qsl], zaf[0:64, :],
                                 rb[0:64, 0, :])
            zs = work.tile([P, 512], BF16, tag="zstage", name="zs", bufs=2)
            nc.vector.tensor_mul(zs[0:64, :], zbf[0:64, :],
                                 rb[0:64, 1, :])
            nc.sync.dma_start(zt[pr][64:128, qsl], zs[0:64, :])

        for pr in range(NPAIR):
            for qc in range(QCN):
                attn(pr, qc, pr * QCN + qc)

        # drain remaining fillers (O projections for the last chunks)
        while fillers:
            fillers.pop()[1]()

    nc.compile()
    return nc



# ---------------- host-side entry point ----------------

BF = ml_dtypes.bfloat16
_B, _S, _D, _H = 4, 2048, 1024, 16
_DQ = _D // 2
_NC_CACHE = None


def _get_nc():
    global _NC_CACHE
    if _NC_CACHE is None:
        _NC_CACHE = build_attention_nc(S_full=_S, D=_D, DQ=_DQ)
    return _NC_CACHE


def kernel(X, Wq, bq, Wk, bk, Wv, bv, Wo, bo):
    """Full-input multi-head attention on 8 TRN2 NeuronCores.

    Sharding: core c handles batch c//2 and head-half c%2 (8 of 16
    heads, column shards of Wq/Wk/Wv, row shard of Wo). Each core
    returns a partial output projection; the host sums the two
    partials per batch and adds bo. Inputs are cast to bf16 on host.
    """
    from concourse.bass_utils import run_bass_kernel_spmd

    X = np.asarray(X, dtype=np.float32)
    bq = np.asarray(bq, dtype=np.float32)
    bk = np.asarray(bk, dtype=np.float32)
    bvb = np.ascontiguousarray(np.asarray(bv, dtype=np.float32).astype(BF))
    bo = np.asarray(bo, dtype=np.float32)
    wqb = np.asarray(Wq, dtype=np.float32).astype(BF)
    wkb = np.asarray(Wk, dtype=np.float32).astype(BF)
    wvb = np.asarray(Wv, dtype=np.float32).astype(BF)
    wob = np.asarray(Wo, dtype=np.float32).astype(BF)

    in_maps = []
    xtb = [np.ascontiguousarray(X[b].T.astype(BF)) for b in range(_B)]
    for c in range(8):
        b, half = c // 2, c % 2
        hsl = slice(half * _DQ, (half + 1) * _DQ)
        in_maps.append({
            "xt": xtb[b],
            "wq": np.ascontiguousarray(wqb[:, hsl]),
            "wk": np.ascontiguousarray(wkb[:, hsl]),
            "wv": np.ascontiguousarray(wvb[:, hsl]),
            "wo": np.ascontiguousarray(wob[hsl, :]),
            "bq": np.ascontiguousarray(bq[hsl]),
            "bk": np.ascontiguousarray(bk[hsl]),
            "bv": np.ascontiguousarray(bvb[hsl]),
        })

    nc = _get_nc()
    res = run_bass_kernel_spmd(nc, in_maps, list(range(8)))

    out = np.empty((_B, _S, _D), np.float32)
    for b in range(_B):
        acc = res.results[2 * b]["ot"] + res.results[2 * b + 1]["ot"]
        out[b] = acc.T + bo
    return out


# revision 15
# speedup vs baseline: 1.0250x; 1.0250x over previous
"""Bass/Tile attention kernel for TRN2 — per-core program builder.

Sharding (v3, tensor-parallel over heads): core c of 8 handles batch
b = c//2 and head-half h = c%2 (8 of 16 heads). Each core projects
Q/K/V for its 8 heads over ALL 2048 tokens (column-shard of Wq/Wk/Wv),
runs attention, and computes a PARTIAL output projection against its
row-shard of Wo. The host sums the two partials per batch and adds bo.
vs. the query-sharded v2 this halves the K/V projection matmul work
(no duplicate K/V proj per query half).

Per-core DRAM inputs:
  xt  : [D, S]   bf16  X[b]^T (full)
  wq/wk/wv : [D, DQ=512] bf16 column shards; wo : [DQ, D] bf16 row shard
  bq/bk : [DQ] fp32; bv : [DQ] bf16
Output:
  ot  : [D, S] fp32  partial O^T (host: out = (ot0+ot1)^T + bo)

Schedule: the kc loop is software-pipelined and emitted in the order
[ACT(kc+1), scores(kc+2), filler, PV(kc)] so that (engines execute
their queues in order) the PE never head-blocks while ScalarE's exp
runs, and projection-chain slices (2 matmuls each, deadline-ordered)
fill the remaining PE slack. PSUM (8 banks): scores 2x[128,2,512]
double-buffered (4), z chains za/zb (2, evicted to SBUF right after
each stream), projection chains double-buffered (2).

Attention per (pair of heads, 512-query chunk), per kc (128 keys):
  scores^T[k,q] both heads -> PSUM [128,2,512] (row-packed matmuls,
  head-lo rows 0-63 / head-hi rows 64-127); probs = exp(0.125*scores)
  as one ACT [128,1024] -> bf16; PV per head with augmented [V_h | 1]
  stationary (M=65) accumulating z + rowsum over kc. Normalization
  divides by the rowsum via a DRAM-bounce partition-broadcast of the
  reciprocal (SBUF->SBUF broadcast DMA is illegal; the custom recip op
  needs fp32 SBUF at base partition 0).
"""

import numpy as np
import ml_dtypes
from contextlib import ExitStack

import concourse.bass as bass
import concourse.tile as tile
from concourse import bacc, mybir

F32 = mybir.dt.float32
BF16 = mybir.dt.bfloat16
P = 128


def build_attention_nc(S_full=2048, D=1024, DQ=512):
    NPAIR = DQ // P           # head pairs (128 dims each) = 4
    KC = S_full // P          # k chunks = 16
    TOKC = S_full // 512      # 512-token chunks (K proj / queries) = 4
    NTOK = S_full // P        # 128-token chunks for V proj = 16
    QCN = S_full // 512       # 512-query chunks = 4
    DIN = D // P              # input-dim chunks = 8
    DCO = DQ // P             # o-proj contraction chunks = 4

    nc = bacc.Bacc("TRN2", target_bir_lowering=False, debug=False)

    xt_d = nc.dram_tensor("xt", [D, S_full], BF16, kind="ExternalInput").ap()
    wq_d = nc.dram_tensor("wq", [D, DQ], BF16, kind="ExternalInput").ap()
    wk_d = nc.dram_tensor("wk", [D, DQ], BF16, kind="ExternalInput").ap()
    wv_d = nc.dram_tensor("wv", [D, DQ], BF16, kind="ExternalInput").ap()
    wo_d = nc.dram_tensor("wo", [DQ, D], BF16, kind="ExternalInput").ap()
    bq_d = nc.dram_tensor("bq", [DQ], F32, kind="ExternalInput").ap()
    bk_d = nc.dram_tensor("bk", [DQ], F32, kind="ExternalInput").ap()
    bv_d = nc.dram_tensor("bv", [DQ], BF16, kind="ExternalInput").ap()
    ot_d = nc.dram_tensor("ot", [D, S_full], BF16, kind="ExternalOutput").ap()

    xt_r = xt_d.rearrange("(c p) t -> p c t", p=P)
    wq_r = wq_d.rearrange("(c p) n -> p c n", p=P)
    wk_r = wk_d.rearrange("(c p) n -> p c n", p=P)
    wv_r = wv_d.rearrange("(c p) n -> p c n", p=P)
    wo_r = wo_d.rearrange("(c p) n -> p c n", p=P)
    bq_r = bq_d.rearrange("(c p) -> p c", p=P)
    bk_r = bk_d.rearrange("(c p) -> p c", p=P)
    bv_r = bv_d.rearrange("(a d) -> a d", a=1)

    EXP = mybir.ActivationFunctionType.Exp

    with tile.TileContext(nc) as tc, ExitStack() as ctx:
        const = ctx.enter_context(tc.tile_pool(name="const", bufs=1))
        big = ctx.enter_context(tc.tile_pool(name="big", bufs=1))
        wpool = ctx.enter_context(tc.tile_pool(name="wpool", bufs=4))
        work = ctx.enter_context(tc.tile_pool(name="work", bufs=3))
        probs_pool = ctx.enter_context(tc.tile_pool(name="probs", bufs=3))
        zsb = ctx.enter_context(tc.tile_pool(name="zsb", bufs=4))
        # PSUM budget (8 banks): scores 2x2 + z 2 + proj 2
        spsum = ctx.enter_context(tc.tile_pool(name="spsum", bufs=2, space="PSUM"))
        zpsum = ctx.enter_context(tc.tile_pool(name="zpsum", bufs=2, space="PSUM"))
        ppsum = ctx.enter_context(tc.tile_pool(name="ppsum", bufs=2, space="PSUM"))
        dramp = ctx.enter_context(tc.tile_pool(name="dramp", bufs=2, space="DRAM"))

        # ---- constants (on the ACT HWDGE ring, off the main input ring)
        bq_t = const.tile([P, NPAIR], F32)
        nc.scalar.dma_start(bq_t[:], bq_r[:, :])
        bk_t = const.tile([P, NPAIR], F32)
        nc.scalar.dma_start(bk_t[:], bk_r[:, :])
        bvb = const.tile([P, DQ], BF16)
        nc.scalar.dma_start(bvb[:], bv_r[0:1, :].to_broadcast((P, DQ)))

        # ---- X^T + weight loads. Ordered so the first attention stream
        # can start ~12us in: wk + first 512-token block of xt, then wq
        # (first scores), then wv (first PV), then the remaining xt
        # blocks (consumed progressively by K/V chains), wo last.
        xt_t = big.tile([P, DIN, S_full], BF16, tag="xt")
        wk_t = wpool.tile([P, DIN, DQ], BF16, tag="w", name="wk_t")
        wq_t = wpool.tile([P, DIN, DQ], BF16, tag="w", name="wq_t")
        wv_t = wpool.tile([P, DIN, DQ], BF16, tag="w", name="wv_t")
        wo_t = wpool.tile([P, DCO, D], BF16, tag="wo", name="wo_t")
        # weights go on the ACT HWDGE ring, xt/wk on the SP ring — the two
        # rings transfer in parallel, halving the serial input-DMA head
        for c in range(DIN):
            nc.sync.dma_start(wk_t[:, c, :], wk_r[:, c, :])
            nc.sync.dma_start(xt_t[:, c, 0:512], xt_r[:, c, 0:512])
        for c in range(DIN):
            nc.scalar.dma_start(wq_t[:, c, :], wq_r[:, c, :])
        for c in range(DIN):
            nc.scalar.dma_start(wv_t[:, c, :], wv_r[:, c, :])
        for tb in range(1, TOKC):
            for c in range(DIN):
                nc.sync.dma_start(xt_t[:, c, tb * 512:(tb + 1) * 512],
                                  xt_r[:, c, tb * 512:(tb + 1) * 512])
        for c in range(DCO):
            nc.scalar.dma_start(wo_t[:, c, :], wo_r[:, c, :])

        kt = [big.tile([P, S_full], BF16, tag=f"kt{p}", name=f"kt{p}")
              for p in range(NPAIR)]
        qt = [big.tile([P, S_full], BF16, tag=f"qt{p}", name=f"qt{p}")
              for p in range(NPAIR)]
        # per-pair augmented V: 2 heads x (64 data + 1 ones col)
        vt = [big.tile([P, NTOK, 130], BF16, tag=f"vt{p}", name=f"vt{p}")
              for p in range(NPAIR)]
        zt = [big.tile([P, S_full], BF16, tag=f"zt{p}", name=f"zt{p}")
              for p in range(NPAIR)]

        for pr in range(NPAIR):
            ones_ap = vt[pr].rearrange("p t (h c) -> p t h c", c=65)[:, :, :, 64:65]
            nc.vector.memset(ones_ap, 1.0)

        # ---- projection chain thunks (each thunk emits ~2 matmuls) ----
        def k_chain(pr, t):
            st = {}
            def mk(c0):
                def f():
                    if c0 == 0:
                        st['ps'] = ppsum.tile([P, 512], F32, tag="proj",
                                              name="psk")
                    for c in (c0, c0 + 1):
                        nc.tensor.matmul(
                            st['ps'][:],
                            wk_t[:, c, pr * P:(pr + 1) * P],
                            xt_t[:, c, t * 512:(t + 1) * 512],
                            start=(c == 0), stop=(c == DIN - 1),
                        )
                    if c0 == DIN - 2:
                        nc.vector.tensor_scalar_add(
                            kt[pr][:, t * 512:(t + 1) * 512], st['ps'][:],
                            bk_t[:, pr:pr + 1])
                return f
            return [mk(c0) for c0 in range(0, DIN, 2)]

        def q_chain(pr, t):
            st = {}
            def mk(c0):
                def f():
                    if c0 == 0:
                        st['ps'] = ppsum.tile([P, 512], F32, tag="proj",
                                              name="psq")
                    for c in (c0, c0 + 1):
                        nc.tensor.matmul(
                            st['ps'][:],
                            wq_t[:, c, pr * P:(pr + 1) * P],
                            xt_t[:, c, t * 512:(t + 1) * 512],
                            start=(c == 0), stop=(c == DIN - 1),
                        )
                    if c0 == DIN - 2:
                        nc.vector.tensor_scalar_add(
                            qt[pr][:, t * 512:(t + 1) * 512], st['ps'][:],
                            bq_t[:, pr:pr + 1])
                return f
            return [mk(c0) for c0 in range(0, DIN, 2)]

        def v_chain(g, t):
            prs = (2 * g, 2 * g + 1)
            st = {}
            def mk(c0):
                def f():
                    if c0 == 0:
                        st['ps'] = ppsum.tile([P, 256], F32, tag="proj",
                                              name="psv")
                    for c in (c0, c0 + 1):
                        nc.tensor.matmul(
                            st['ps'][:],
                            xt_t[:, c, t * P:(t + 1) * P],
                            wv_t[:, c, g * 256:(g + 1) * 256],
                            start=(c == 0), stop=(c == DIN - 1),
                        )
                    if c0 == DIN - 2:
                        for u in range(2):
                            dst = vt[prs[u]].rearrange(
                                "p t (h c) -> p t h c", c=65)[:, t, :, 0:64]
                            src = st['ps'][:, u * 128:(u + 1) * 128].rearrange(
                                "p (h c) -> p h c", c=64)
                            bsrc = bvb[:, (g * 256 + u * 128):
                                       (g * 256 + (u + 1) * 128)
                                       ].rearrange("p (h c) -> p h c", c=64)
                            nc.vector.tensor_add(dst, src, bsrc)
                return f
            return [mk(c0) for c0 in range(0, DIN, 2)]

        def o_chain(qc, ec):
            oqsl = slice(qc * 512, (qc + 1) * 512)
            st = {}
            def mk(d0):
                def f():
                    if d0 == 0:
                        st['ps'] = ppsum.tile([P, 512], F32, tag="proj",
                                              name="pso")
                    for dc in (d0, d0 + 1):
                        nc.tensor.matmul(
                            st['ps'][:],
                            wo_t[:, dc, ec * P:(ec + 1) * P],
                            zt[dc][:, oqsl],
                            start=(dc == 0), stop=(dc == DCO - 1),
                        )
                    if d0 == DCO - 2:
                        stg = work.tile([P, 512], BF16, tag="stage",
                                        name="st", bufs=2)
                        nc.vector.tensor_copy(stg[:], st['ps'][:])
                        nc.sync.dma_start(
                            ot_d[ec * P:(ec + 1) * P, oqsl], stg[:])
                return f
            return [mk(d0) for d0 in range(0, DCO, 2)]

        # ---- filler queue: (deadline_stream, thunk), emitted in order ----
        fillers = []

        def push(deadline, thunks):
            for th in thunks:
                fillers.append((deadline, th))

        # head work (before stream 0): K(0,t0), Q(0,qc0), V(quad0) t0/t1;
        # the rest of K(0)/V(quad0) is emitted inside stream 0 (V chains
        # just-in-time, one per kc, two iterations ahead of the PV that
        # consumes them).
        for th in k_chain(0, 0):
            th()
        for th in q_chain(0, 0):
            th()
        for th in v_chain(0, 0):
            th()
        for th in v_chain(0, 1):
            th()

        # stream s = pr*4 + qc; deadline = stream during which to emit.
        # Every producer's deadline is < its first consumer stream; loads
        # are spread so budget-2 pops drain each stream's quota in-stream.
        for t in range(1, TOKC):
            push(0, k_chain(0, t))
        push(0, q_chain(0, 1))
        push(1, k_chain(1, 0)); push(1, k_chain(1, 1))
        push(1, q_chain(0, 2))
        push(2, k_chain(1, 2)); push(2, k_chain(1, 3))
        push(2, q_chain(0, 3))
        push(3, q_chain(1, 0))
        for t in range(NTOK):
            push(4 + t // 4, v_chain(1, t))
        push(4, q_chain(1, 1))
        push(5, q_chain(1, 2))
        push(6, k_chain(2, 0)); push(6, k_chain(2, 1))
        push(6, q_chain(1, 3))
        push(7, k_chain(2, 2)); push(7, k_chain(2, 3))
        push(7, q_chain(2, 0))
        push(8, q_chain(2, 1))
        push(9, q_chain(2, 2))
        push(10, k_chain(3, 0)); push(10, k_chain(3, 1))
        push(10, q_chain(2, 3))
        push(11, k_chain(3, 2)); push(11, k_chain(3, 3))
        push(11, q_chain(3, 0))
        push(12, q_chain(3, 1))
        push(13, [th for ec in range(DIN) for th in o_chain(0, ec)])
        push(13, q_chain(3, 2))
        push(14, [th for ec in range(DIN) for th in o_chain(1, ec)])
        push(14, q_chain(3, 3))
        push(15, [th for ec in range(DIN) for th in o_chain(2, ec)])
        push(16, [th for ec in range(DIN) for th in o_chain(3, ec)])

        fillers.sort(key=lambda x: x[0])
        fillers = list(reversed(fillers))  # pop from end

        def pop_fillers(s, budget):
            n = 0
            while fillers and (fillers[-1][0] < s
                               or (fillers[-1][0] <= s and n < budget)):
                fillers.pop()[1]()
                n += 1

        # ---- one attention stream: (pair pr, 512-query chunk qc) ----
        def attn(pr, qc, s):
            # Emission order IS program semantics (Tile serializes a
            # later-emitted write AFTER an earlier-emitted read of the same
            # region), so every producer chain with deadline < s must be
            # fully emitted before this stream's first scores read.
            pop_fillers(s, 0)
            vpr = vt[pr].rearrange("p t (h c) -> p t h c", c=65)
            qsl = slice(qc * 512, (qc + 1) * 512)
            za = zpsum.tile([P, 512], F32, tag="z", name=f"za{pr}_{qc}")
            zb = zpsum.tile([P, 512], F32, tag="z", name=f"zb{pr}_{qc}")

            def emit_scores(kc):
                sq = spsum.tile([P, 2, 512], F32, tag="sc", name="sq")
                nc.tensor.matmul(
                    sq[:, 0, :],
                    kt[pr][0:64, kc * P:(kc + 1) * P],
                    qt[pr][0:64, qsl],
                    start=True, stop=True,
                )
                nc.tensor.matmul(
                    sq[:, 1, :],
                    kt[pr][64:128, kc * P:(kc + 1) * P],
                    qt[pr][64:128, qsl],
                    start=True, stop=True,
                )
                return sq

            def emit_act(sq):
                pq = probs_pool.tile([P, 2, 512], BF16, tag="probs",
                                     name="pq")
                nc.scalar.activation(pq[:], sq[:], EXP, scale=0.125)
                return pq

            def emit_pv(kc, pq):
                nc.tensor.matmul(
                    za[0:65, :], vpr[:, kc, 0, :], pq[:, 0, :],
                    start=(kc == 0), stop=(kc == KC - 1),
                )
                nc.tensor.matmul(
                    zb[0:65, :], vpr[:, kc, 1, :], pq[:, 1, :],
                    start=(kc == 0), stop=(kc == KC - 1),
                )

            # software pipeline: ACT(kc+1), scores(kc+2), fillers, PV(kc)
            sq0 = emit_scores(0)
            pq_cur = emit_act(sq0)
            sq_next = emit_scores(1)
            for kc in range(KC):
                pq_next = None
                if kc + 1 < KC:
                    pq_next = emit_act(sq_next)
                if kc + 2 < KC:
                    sq_next = emit_scores(kc + 2)
                pop_fillers(s, 2)
                emit_pv(kc, pq_cur)
                if s == 0 and kc + 2 < KC:
                    # just-in-time V(quad0) chain for kc+2, two iterations
                    # ahead of the PV that reads it
                    for th in v_chain(0, kc + 2):
                        th()
                pq_cur = pq_next

            # evict z chains to SBUF (frees the z banks), then normalize
            zaf = zsb.tile([P, 512], F32, tag="zf", name="zaf")
            nc.vector.tensor_copy(zaf[0:65, :], za[0:65, :])
            zbf = zsb.tile([P, 512], F32, tag="zf", name="zbf")
            nc.vector.tensor_copy(zbf[0:65, :], zb[0:65, :])
            rsc = dramp.tile([2, 512], F32, tag="rsc", name="rsc")
            nc.sync.dma_start(rsc[0:1, :], zaf[64:65, :])
            nc.sync.dma_start(rsc[1:2, :], zbf[64:65, :])
            rbr = work.tile([P, 2, 512], F32, tag="rbc", name="rbr", bufs=1)
            nc.sync.dma_start(rbr[0:64, 0, :],
                              rsc[0:1, :].to_broadcast((64, 512)))
            nc.sync.dma_start(rbr[0:64, 1, :],
                              rsc[1:2, :].to_broadcast((64, 512)))
            rb = work.tile([P, 2, 512], F32, tag="rbc2", name="rb", bufs=1)
            nc.vector.reciprocal_approx_fast(rb[0:64, :, :],
                                             rbr[0:64, :, :])
            nc.vector.tensor_mul(zt[pr][0:64, qsl], zaf[0:64, :],
                                 rb[0:64, 0, :])
            zs = work.tile([P, 512], BF16, tag="zstage", name="zs", bufs=2)
            nc.vector.tensor_mul(zs[0:64, :], zbf[0:64, :],
                                 rb[0:64, 1, :])
            nc.sync.dma_start(zt[pr][64:128, qsl], zs[0:64, :])

        for pr in range(NPAIR):
            for qc in range(QCN):
                attn(pr, qc, pr * QCN + qc)

        # drain remaining fillers (O projections for the last chunks)
        while fillers:
            fillers.pop()[1]()

    nc.compile()
    return nc



# ---------------- host-side entry point ----------------

BF = ml_dtypes.bfloat16
_B, _S, _D, _H = 4, 2048, 1024, 16
_DQ = _D // 2
_NC_CACHE = None


def _get_nc():
    global _NC_CACHE
    if _NC_CACHE is None:
        _NC_CACHE = build_attention_nc(S_full=_S, D=_D, DQ=_DQ)
    return _NC_CACHE


def kernel(X, Wq, bq, Wk, bk, Wv, bv, Wo, bo):
    """Full-input multi-head attention on 8 TRN2 NeuronCores.

    Sharding: core c handles batch c//2 and head-half c%2 (8 of 16
    heads, column shards of Wq/Wk/Wv, row shard of Wo). Each core
    returns a partial output projection; the host sums the two
    partials per batch and adds bo. Inputs are cast to bf16 on host.
    """
    from concourse.bass_utils import run_bass_kernel_spmd

    X = np.asarray(X, dtype=np.float32)
    bq = np.asarray(bq, dtype=np.float32)
    bk = np.asarray(bk, dtype=np.float32)
    bvb = np.ascontiguousarray(np.asarray(bv, dtype=np.float32).astype(BF))
    bo = np.asarray(bo, dtype=np.float32)
    wqb = np.asarray(Wq, dtype=np.float32).astype(BF)
    wkb = np.asarray(Wk, dtype=np.float32).astype(BF)
    wvb = np.asarray(Wv, dtype=np.float32).astype(BF)
    wob = np.asarray(Wo, dtype=np.float32).astype(BF)

    in_maps = []
    xtb = [np.ascontiguousarray(X[b].T.astype(BF)) for b in range(_B)]
    for c in range(8):
        b, half = c // 2, c % 2
        hsl = slice(half * _DQ, (half + 1) * _DQ)
        in_maps.append({
            "xt": xtb[b],
            "wq": np.ascontiguousarray(wqb[:, hsl]),
            "wk": np.ascontiguousarray(wkb[:, hsl]),
            "wv": np.ascontiguousarray(wvb[:, hsl]),
            "wo": np.ascontiguousarray(wob[hsl, :]),
            "bq": np.ascontiguousarray(bq[hsl]),
            "bk": np.ascontiguousarray(bk[hsl]),
            "bv": np.ascontiguousarray(bvb[hsl]),
        })

    nc = _get_nc()
    res = run_bass_kernel_spmd(nc, in_maps, list(range(8)))

    out = np.empty((_B, _S, _D), np.float32)
    for b in range(_B):
        acc = (res.results[2 * b]["ot"].astype(np.float32)
               + res.results[2 * b + 1]["ot"].astype(np.float32))
        out[b] = acc.T + bo
    return out


# revision 21
# speedup vs baseline: 1.0265x; 1.0015x over previous
"""Bass/Tile attention kernel for TRN2 — per-core program builder.

Sharding (v3, tensor-parallel over heads): core c of 8 handles batch
b = c//2 and head-half h = c%2 (8 of 16 heads). Each core projects
Q/K/V for its 8 heads over ALL 2048 tokens (column-shard of Wq/Wk/Wv),
runs attention, and computes a PARTIAL output projection against its
row-shard of Wo. The host sums the two partials per batch and adds bo.
vs. the query-sharded v2 this halves the K/V projection matmul work
(no duplicate K/V proj per query half).

Per-core DRAM inputs:
  xt  : [D, S]   bf16  X[b]^T (full)
  wq/wk/wv : [D, DQ=512] bf16 column shards; wo : [DQ, D] bf16 row shard
  bq/bk : [DQ] fp32; bv : [DQ] bf16
Output:
  ot  : [D, S] fp32  partial O^T (host: out = (ot0+ot1)^T + bo)

Schedule: the kc loop is software-pipelined and emitted in the order
[ACT(kc+1), scores(kc+2), filler, PV(kc)] so that (engines execute
their queues in order) the PE never head-blocks while ScalarE's exp
runs, and projection-chain slices (2 matmuls each, deadline-ordered)
fill the remaining PE slack. PSUM (8 banks): scores 2x[128,2,512]
double-buffered (4), z chains za/zb (2, evicted to SBUF right after
each stream), projection chains double-buffered (2).

Attention per (pair of heads, 512-query chunk), per kc (128 keys):
  scores^T[k,q] both heads -> PSUM [128,2,512] (row-packed matmuls,
  head-lo rows 0-63 / head-hi rows 64-127); probs = exp(0.125*scores)
  as one ACT [128,1024] -> bf16; PV per head with augmented [V_h | 1]
  stationary (M=65) accumulating z + rowsum over kc. Normalization
  divides by the rowsum via a DRAM-bounce partition-broadcast of the
  reciprocal (SBUF->SBUF broadcast DMA is illegal; the custom recip op
  needs fp32 SBUF at base partition 0).
"""

import numpy as np
import ml_dtypes
from contextlib import ExitStack

import concourse.bass as bass
import concourse.tile as tile
from concourse import bacc, mybir

F32 = mybir.dt.float32
BF16 = mybir.dt.bfloat16
P = 128


def build_attention_nc(S_full=2048, D=1024, DQ=512):
    NPAIR = DQ // P           # head pairs (128 dims each) = 4
    KC = S_full // P          # k chunks = 16
    TOKC = S_full // 512      # 512-token chunks (K proj / queries) = 4
    NTOK = S_full // P        # 128-token chunks for V proj = 16
    QCN = S_full // 512       # 512-query chunks = 4
    DIN = D // P              # input-dim chunks = 8
    DCO = DQ // P             # o-proj contraction chunks = 4

    nc = bacc.Bacc("TRN2", target_bir_lowering=False, debug=False)

    xt_d = nc.dram_tensor("xt", [D, S_full], BF16, kind="ExternalInput").ap()
    wq_d = nc.dram_tensor("wq", [D, DQ], BF16, kind="ExternalInput").ap()
    wk_d = nc.dram_tensor("wk", [D, DQ], BF16, kind="ExternalInput").ap()
    wv_d = nc.dram_tensor("wv", [D, DQ], BF16, kind="ExternalInput").ap()
    wo_d = nc.dram_tensor("wo", [DQ, D], BF16, kind="ExternalInput").ap()
    bq_d = nc.dram_tensor("bq", [DQ], F32, kind="ExternalInput").ap()
    bk_d = nc.dram_tensor("bk", [DQ], F32, kind="ExternalInput").ap()
    bv_d = nc.dram_tensor("bv", [DQ], BF16, kind="ExternalInput").ap()
    ot_d = nc.dram_tensor("ot", [D, S_full], BF16, kind="ExternalOutput").ap()

    xt_r = xt_d.rearrange("(c p) t -> p c t", p=P)
    wq_r = wq_d.rearrange("(c p) n -> p c n", p=P)
    wk_r = wk_d.rearrange("(c p) n -> p c n", p=P)
    wv_r = wv_d.rearrange("(c p) n -> p c n", p=P)
    wo_r = wo_d.rearrange("(c p) n -> p c n", p=P)
    bq_r = bq_d.rearrange("(c p) -> p c", p=P)
    bk_r = bk_d.rearrange("(c p) -> p c", p=P)
    bv_r = bv_d.rearrange("(a d) -> a d", a=1)

    EXP = mybir.ActivationFunctionType.Exp

    with tile.TileContext(nc) as tc, ExitStack() as ctx:
        const = ctx.enter_context(tc.tile_pool(name="const", bufs=1))
        big = ctx.enter_context(tc.tile_pool(name="big", bufs=1))
        wpool = ctx.enter_context(tc.tile_pool(name="wpool", bufs=4))
        work = ctx.enter_context(tc.tile_pool(name="work", bufs=3))
        probs_pool = ctx.enter_context(tc.tile_pool(name="probs", bufs=3))
        zsb = ctx.enter_context(tc.tile_pool(name="zsb", bufs=4))
        # PSUM budget (8 banks): scores 2x2 + z 2 + proj 2
        spsum = ctx.enter_context(tc.tile_pool(name="spsum", bufs=2, space="PSUM"))
        zpsum = ctx.enter_context(tc.tile_pool(name="zpsum", bufs=2, space="PSUM"))
        ppsum = ctx.enter_context(tc.tile_pool(name="ppsum", bufs=2, space="PSUM"))
        dramp = ctx.enter_context(tc.tile_pool(name="dramp", bufs=2, space="DRAM"))

        # ---- constants (on the ACT HWDGE ring, off the main input ring)
        bq_t = const.tile([P, NPAIR], F32)
        nc.scalar.dma_start(bq_t[:], bq_r[:, :])
        bk_t = const.tile([P, NPAIR], F32)
        nc.scalar.dma_start(bk_t[:], bk_r[:, :])
        bvb = const.tile([P, DQ], BF16)
        nc.scalar.dma_start(bvb[:], bv_r[0:1, :].to_broadcast((P, DQ)))

        # ---- X^T + weight loads. Ordered so the first attention stream
        # can start ~12us in: wk + first 512-token block of xt, then wq
        # (first scores), then wv (first PV), then the remaining xt
        # blocks (consumed progressively by K/V chains), wo last.
        xt_t = big.tile([P, DIN, S_full], BF16, tag="xt")
        wk_t = wpool.tile([P, DIN, DQ], BF16, tag="w", name="wk_t")
        wq_t = wpool.tile([P, DIN, DQ], BF16, tag="w", name="wq_t")
        wv_t = wpool.tile([P, DIN, DQ], BF16, tag="w", name="wv_t")
        wo_t = wpool.tile([P, DCO, D], BF16, tag="wo", name="wo_t")
        # weights go on the ACT HWDGE ring, xt/wk on the SP ring — the two
        # rings transfer in parallel, halving the serial input-DMA head
        for c in range(DIN):
            nc.sync.dma_start(wk_t[:, c, :], wk_r[:, c, :])
            nc.sync.dma_start(xt_t[:, c, 0:512], xt_r[:, c, 0:512])
        for c in range(DIN):
            nc.scalar.dma_start(wq_t[:, c, :], wq_r[:, c, :])
        for c in range(DIN):
            nc.scalar.dma_start(wv_t[:, c, :], wv_r[:, c, :])
        for tb in range(1, TOKC):
            for c in range(DIN):
                nc.sync.dma_start(xt_t[:, c, tb * 512:(tb + 1) * 512],
                                  xt_r[:, c, tb * 512:(tb + 1) * 512])
        for c in range(DCO):
            nc.scalar.dma_start(wo_t[:, c, :], wo_r[:, c, :])

        kt = [big.tile([P, S_full], BF16, tag=f"kt{p}", name=f"kt{p}")
              for p in range(NPAIR)]
        qt = [big.tile([P, S_full], BF16, tag=f"qt{p}", name=f"qt{p}")
              for p in range(NPAIR)]
        # per-pair augmented V: 2 heads x (64 data + 1 ones col)
        vt = [big.tile([P, NTOK, 130], BF16, tag=f"vt{p}", name=f"vt{p}")
              for p in range(NPAIR)]
        zt = [big.tile([P, S_full], BF16, tag=f"zt{p}", name=f"zt{p}")
              for p in range(NPAIR)]

        for pr in range(NPAIR):
            ones_ap = vt[pr].rearrange("p t (h c) -> p t h c", c=65)[:, :, :, 64:65]
            nc.vector.memset(ones_ap, 1.0)

        # ---- projection chain thunks (each thunk emits ~2 matmuls) ----
        def k_chain(pr, t):
            st = {}
            def mk(c0):
                def f():
                    if c0 == 0:
                        st['ps'] = ppsum.tile([P, 512], F32, tag="proj",
                                              name="psk")
                    for c in (c0, c0 + 1):
                        nc.tensor.matmul(
                            st['ps'][:],
                            wk_t[:, c, pr * P:(pr + 1) * P],
                            xt_t[:, c, t * 512:(t + 1) * 512],
                            start=(c == 0), stop=(c == DIN - 1),
                        )
                    if c0 == DIN - 2:
                        nc.vector.tensor_scalar_add(
                            kt[pr][:, t * 512:(t + 1) * 512], st['ps'][:],
                            bk_t[:, pr:pr + 1])
                return f
            return [mk(c0) for c0 in range(0, DIN, 2)]

        def q_chain(pr, t):
            st = {}
            def mk(c0):
                def f():
                    if c0 == 0:
                        st['ps'] = ppsum.tile([P, 512], F32, tag="proj",
                                              name="psq")
                    for c in (c0, c0 + 1):
                        nc.tensor.matmul(
                            st['ps'][:],
                            wq_t[:, c, pr * P:(pr + 1) * P],
                            xt_t[:, c, t * 512:(t + 1) * 512],
                            start=(c == 0), stop=(c == DIN - 1),
                        )
                    if c0 == DIN - 2:
                        nc.vector.tensor_scalar_add(
                            qt[pr][:, t * 512:(t + 1) * 512], st['ps'][:],
                            bq_t[:, pr:pr + 1])
                return f
            return [mk(c0) for c0 in range(0, DIN, 2)]

        def v_chain(g, t):
            prs = (2 * g, 2 * g + 1)
            st = {}
            def mk(c0):
                def f():
                    if c0 == 0:
                        st['ps'] = ppsum.tile([P, 256], F32, tag="proj",
                                              name="psv")
                    for c in (c0, c0 + 1):
                        nc.tensor.matmul(
                            st['ps'][:],
                            xt_t[:, c, t * P:(t + 1) * P],
                            wv_t[:, c, g * 256:(g + 1) * 256],
                            start=(c == 0), stop=(c == DIN - 1),
                        )
                    if c0 == DIN - 2:
                        for u in range(2):
                            dst = vt[prs[u]].rearrange(
                                "p t (h c) -> p t h c", c=65)[:, t, :, 0:64]
                            src = st['ps'][:, u * 128:(u + 1) * 128].rearrange(
                                "p (h c) -> p h c", c=64)
                            bsrc = bvb[:, (g * 256 + u * 128):
                                       (g * 256 + (u + 1) * 128)
                                       ].rearrange("p (h c) -> p h c", c=64)
                            nc.vector.tensor_add(dst, src, bsrc)
                return f
            return [mk(c0) for c0 in range(0, DIN, 2)]

        def o_chain(qc, ec):
            oqsl = slice(qc * 512, (qc + 1) * 512)
            st = {}
            def mk(d0):
                def f():
                    if d0 == 0:
                        st['ps'] = ppsum.tile([P, 512], F32, tag="proj",
                                              name="pso")
                    for dc in (d0, d0 + 1):
                        nc.tensor.matmul(
                            st['ps'][:],
                            wo_t[:, dc, ec * P:(ec + 1) * P],
                            zt[dc][:, oqsl],
                            start=(dc == 0), stop=(dc == DCO - 1),
                        )
                    if d0 == DCO - 2:
                        stg = work.tile([P, 512], BF16, tag="stage",
                                        name="st", bufs=2)
                        nc.vector.tensor_copy(stg[:], st['ps'][:])
                        nc.sync.dma_start(
                            ot_d[ec * P:(ec + 1) * P, oqsl], stg[:])
                return f
            return [mk(d0) for d0 in range(0, DCO, 2)]

        # ---- filler queue: (deadline_stream, thunk), emitted in order ----
        fillers = []

        def push(deadline, thunks):
            for th in thunks:
                fillers.append((deadline, th))

        # head work (before stream 0): K(0,t0), Q(0,qc0), V(quad0) t0/t1;
        # the rest of K(0)/V(quad0) is emitted inside stream 0 (V chains
        # just-in-time, one per kc, two iterations ahead of the PV that
        # consumes them).
        for th in k_chain(0, 0):
            th()
        for th in q_chain(0, 0):
            th()
        for th in v_chain(0, 0):
            th()
        for th in v_chain(0, 1):
            th()

        # stream s = pr*4 + qc; deadline = stream during which to emit.
        # Every producer's deadline is < its first consumer stream; loads
        # are spread so budget-2 pops drain each stream's quota in-stream.
        for t in range(1, TOKC):
            push(0, k_chain(0, t))
        push(0, q_chain(0, 1))
        push(1, k_chain(1, 0)); push(1, k_chain(1, 1))
        push(1, q_chain(0, 2))
        push(2, k_chain(1, 2)); push(2, k_chain(1, 3))
        push(2, q_chain(0, 3))
        push(3, q_chain(1, 0))
        for t in range(NTOK):
            push(4 + t // 4, v_chain(1, t))
        push(4, q_chain(1, 1))
        push(5, q_chain(1, 2))
        push(6, k_chain(2, 0)); push(6, k_chain(2, 1))
        push(6, q_chain(1, 3))
        push(7, k_chain(2, 2)); push(7, k_chain(2, 3))
        push(7, q_chain(2, 0))
        push(8, q_chain(2, 1))
        push(9, q_chain(2, 2))
        push(10, k_chain(3, 0)); push(10, k_chain(3, 1))
        push(10, q_chain(2, 3))
        push(11, k_chain(3, 2)); push(11, k_chain(3, 3))
        push(11, q_chain(3, 0))
        push(12, q_chain(3, 1))
        push(13, [th for ec in range(DIN) for th in o_chain(0, ec)])
        push(13, q_chain(3, 2))
        push(14, [th for ec in range(DIN) for th in o_chain(1, ec)])
        push(14, q_chain(3, 3))
        push(15, [th for ec in range(DIN) for th in o_chain(2, ec)])
        push(16, [th for ec in range(DIN) for th in o_chain(3, ec)])

        fillers.sort(key=lambda x: x[0])
        fillers = list(reversed(fillers))  # pop from end

        def pop_fillers(s, budget):
            n = 0
            while fillers and (fillers[-1][0] < s
                               or (fillers[-1][0] <= s and n < budget)):
                fillers.pop()[1]()
                n += 1

        # ---- one attention stream: (pair pr, 512-query chunk qc) ----
        def attn(pr, qc, s):
            # Emission order IS program semantics (Tile serializes a
            # later-emitted write AFTER an earlier-emitted read of the same
            # region), so every producer chain with deadline < s must be
            # fully emitted before this stream's first scores read.
            pop_fillers(s, 0)
            vpr = vt[pr].rearrange("p t (h c) -> p t h c", c=65)
            qsl = slice(qc * 512, (qc + 1) * 512)
            za = zpsum.tile([P, 512], F32, tag="z", name=f"za{pr}_{qc}")
            zb = zpsum.tile([P, 512], F32, tag="z", name=f"zb{pr}_{qc}")

            def emit_scores(kc):
                sq = spsum.tile([P, 2, 512], F32, tag="sc", name="sq")
                nc.tensor.matmul(
                    sq[:, 0, :],
                    kt[pr][0:64, kc * P:(kc + 1) * P],
                    qt[pr][0:64, qsl],
                    start=True, stop=True,
                )
                nc.tensor.matmul(
                    sq[:, 1, :],
                    kt[pr][64:128, kc * P:(kc + 1) * P],
                    qt[pr][64:128, qsl],
                    start=True, stop=True,
                )
                return sq

            def emit_act(sq):
                pq = probs_pool.tile([P, 2, 512], BF16, tag="probs",
                                     name="pq")
                nc.scalar.activation(pq[:], sq[:], EXP, scale=0.125)
                return pq

            def emit_pv(kc, pq):
                nc.tensor.matmul(
                    za[0:65, :], vpr[:, kc, 0, :], pq[:, 0, :],
                    start=(kc == 0), stop=(kc == KC - 1),
                )
                nc.tensor.matmul(
                    zb[0:65, :], vpr[:, kc, 1, :], pq[:, 1, :],
                    start=(kc == 0), stop=(kc == KC - 1),
                )

            # software pipeline: ACT(kc+1), scores(kc+2), fillers, PV(kc)
            sq0 = emit_scores(0)
            pq_cur = emit_act(sq0)
            sq_next = emit_scores(1)
            for kc in range(KC):
                pq_next = None
                if kc + 1 < KC:
                    pq_next = emit_act(sq_next)
                if kc + 2 < KC:
                    sq_next = emit_scores(kc + 2)
                pop_fillers(s, 2)
                emit_pv(kc, pq_cur)
                if s == 0 and kc + 2 < KC:
                    # just-in-time V(quad0) chain for kc+2, two iterations
                    # ahead of the PV that reads it
                    for th in v_chain(0, kc + 2):
                        th()
                pq_cur = pq_next

            # evict z chains to SBUF (frees the z banks), then normalize
            zaf = zsb.tile([P, 512], F32, tag="zf", name="zaf")
            nc.vector.tensor_copy(zaf[0:65, :], za[0:65, :])
            zbf = zsb.tile([P, 512], F32, tag="zf", name="zbf")
            nc.vector.tensor_copy(zbf[0:65, :], zb[0:65, :])
            rsc = dramp.tile([2, 512], F32, tag="rsc", name="rsc")
            nc.sync.dma_start(rsc[0:1, :], zaf[64:65, :])
            nc.sync.dma_start(rsc[1:2, :], zbf[64:65, :])
            rbr = work.tile([P, 2, 512], F32, tag="rbc", name="rbr", bufs=1)
            nc.sync.dma_start(rbr[0:64, 0, :],
                              rsc[0:1, :].to_broadcast((64, 512)))
            nc.sync.dma_start(rbr[0:64, 1, :],
                              rsc[1:2, :].to_broadcast((64, 512)))
            rb = work.tile([P, 2, 512], F32, tag="rbc2", name="rb", bufs=1)
            nc.vector.reciprocal_approx_fast(rb[0:64, :, :],
                                             rbr[0:64, :, :])
            nc.vector.tensor_mul(zt[pr][0:64, qsl], zaf[0:64, :],
                                 rb[0:64, 0, :])
            zs = work.tile([P, 512], BF16, tag="zstage", name="zs", bufs=2)
            nc.vector.tensor_mul(zs[0:64, :], zbf[0:64, :],
                                 rb[0:64, 1, :])
            nc.sync.dma_start(zt[pr][64:128, qsl], zs[0:64, :])

        for pr in range(NPAIR):
            for qc in range(QCN):
                attn(pr, qc, pr * QCN + qc)

        # drain remaining fillers (O projections for the last chunks)
        while fillers:
            fillers.pop()[1]()

    nc.compile()
    return nc



# ---------------- host-side entry point ----------------

BF = ml_dtypes.bfloat16
_B, _S, _D, _H = 4, 2048, 1024, 16
_DQ = _D // 2
_NC_CACHE = None


def _get_nc():
    global _NC_CACHE
    if _NC_CACHE is None:
        _NC_CACHE = build_attention_nc(S_full=_S, D=_D, DQ=_DQ)
    return _NC_CACHE


def kernel(X, Wq, bq, Wk, bk, Wv, bv, Wo, bo):
    """Full-input multi-head attention on 8 TRN2 NeuronCores.

    Sharding: core c handles batch c//2 and head-half c%2 (8 of 16
    heads, column shards of Wq/Wk/Wv, row shard of Wo). Each core
    returns a partial output projection; the host sums the two
    partials per batch and adds bo. Inputs are cast to bf16 on host.
    """
    from concourse.bass_utils import run_bass_kernel_spmd

    X = np.asarray(X, dtype=np.float32)
    bq = np.asarray(bq, dtype=np.float32)
    bk = np.asarray(bk, dtype=np.float32)
    bvb = np.ascontiguousarray(np.asarray(bv, dtype=np.float32).astype(BF))
    bo = np.asarray(bo, dtype=np.float32)
    wqb = np.asarray(Wq, dtype=np.float32).astype(BF)
    wkb = np.asarray(Wk, dtype=np.float32).astype(BF)
    wvb = np.asarray(Wv, dtype=np.float32).astype(BF)
    wob = np.asarray(Wo, dtype=np.float32).astype(BF)

    in_maps = []
    xtb = [np.ascontiguousarray(X[b].T.astype(BF)) for b in range(_B)]
    for c in range(8):
        b, half = c // 2, c % 2
        hsl = slice(half * _DQ, (half + 1) * _DQ)
        in_maps.append({
            "xt": xtb[b],
            "wq": np.ascontiguousarray(wqb[:, hsl]),
            "wk": np.ascontiguousarray(wkb[:, hsl]),
            "wv": np.ascontiguousarray(wvb[:, hsl]),
            "wo": np.ascontiguousarray(wob[hsl, :]),
            "bq": np.ascontiguousarray(bq[hsl]),
            "bk": np.ascontiguousarray(bk[hsl]),
            "bv": np.ascontiguousarray(bvb[hsl]),
        })

    nc = _get_nc()
    res = run_bass_kernel_spmd(nc, in_maps, list(range(8)))

    out = np.empty((_B, _S, _D), np.float32)
    for b in range(_B):
        acc = (res.results[2 * b]["ot"].astype(np.float32)
               + res.results[2 * b + 1]["ot"].astype(np.float32))
        out[b] = acc.T + bo
    return out


# revision 23
# speedup vs baseline: 1.0297x; 1.0031x over previous
"""Bass/Tile attention kernel for TRN2 — per-core program builder.

Sharding (v3, tensor-parallel over heads): core c of 8 handles batch
b = c//2 and head-half h = c%2 (8 of 16 heads). Each core projects
Q/K/V for its 8 heads over ALL 2048 tokens (column-shard of Wq/Wk/Wv),
runs attention, and computes a PARTIAL output projection against its
row-shard of Wo. The host sums the two partials per batch and adds bo.
vs. the query-sharded v2 this halves the K/V projection matmul work
(no duplicate K/V proj per query half).

Per-core DRAM inputs:
  xt  : [D, S]   bf16  X[b]^T (full)
  wq/wk/wv : [D, DQ=512] bf16 column shards; wo : [DQ, D] bf16 row shard
  bq/bk : [DQ] fp32; bv : [DQ] bf16
Output:
  ot  : [D, S] fp32  partial O^T (host: out = (ot0+ot1)^T + bo)

Schedule: the kc loop is software-pipelined and emitted in the order
[ACT(kc+1), scores(kc+2), filler, PV(kc)] so that (engines execute
their queues in order) the PE never head-blocks while ScalarE's exp
runs, and projection-chain slices (2 matmuls each, deadline-ordered)
fill the remaining PE slack. PSUM (8 banks): scores 2x[128,2,512]
double-buffered (4), z chains za/zb (2, evicted to SBUF right after
each stream), projection chains double-buffered (2).

Attention per (pair of heads, 512-query chunk), per kc (128 keys):
  scores^T[k,q] both heads -> PSUM [128,2,512] (row-packed matmuls,
  head-lo rows 0-63 / head-hi rows 64-127); probs = exp(0.125*scores)
  as one ACT [128,1024] -> bf16; PV per head with augmented [V_h | 1]
  stationary (M=65) accumulating z + rowsum over kc. Normalization
  divides by the rowsum via a DRAM-bounce partition-broadcast of the
  reciprocal (SBUF->SBUF broadcast DMA is illegal; the custom recip op
  needs fp32 SBUF at base partition 0).
"""

import numpy as np
import ml_dtypes
from contextlib import ExitStack

import concourse.bass as bass
import concourse.tile as tile
from concourse import bacc, mybir

F32 = mybir.dt.float32
BF16 = mybir.dt.bfloat16
P = 128


def build_attention_nc(S_full=2048, D=1024, DQ=512):
    NPAIR = DQ // P           # head pairs (128 dims each) = 4
    KC = S_full // P          # k chunks = 16
    TOKC = S_full // 512      # 512-token chunks (K proj / queries) = 4
    NTOK = S_full // P        # 128-token chunks for V proj = 16
    QCN = S_full // 512       # 512-query chunks = 4
    DIN = D // P              # input-dim chunks = 8
    DCO = DQ // P             # o-proj contraction chunks = 4

    nc = bacc.Bacc("TRN2", target_bir_lowering=False, debug=False)

    xt_d = nc.dram_tensor("xt", [D, S_full], BF16, kind="ExternalInput").ap()
    wq_d = nc.dram_tensor("wq", [D, DQ], BF16, kind="ExternalInput").ap()
    wk_d = nc.dram_tensor("wk", [D, DQ], BF16, kind="ExternalInput").ap()
    wv_d = nc.dram_tensor("wv", [D, DQ], BF16, kind="ExternalInput").ap()
    wo_d = nc.dram_tensor("wo", [DQ, D], BF16, kind="ExternalInput").ap()
    bq_d = nc.dram_tensor("bq", [DQ], F32, kind="ExternalInput").ap()
    bk_d = nc.dram_tensor("bk", [DQ], F32, kind="ExternalInput").ap()
    bv_d = nc.dram_tensor("bv", [DQ], BF16, kind="ExternalInput").ap()
    ot_d = nc.dram_tensor("ot", [D, S_full], BF16, kind="ExternalOutput").ap()

    xt_r = xt_d.rearrange("(c p) t -> p c t", p=P)
    wq_r = wq_d.rearrange("(c p) n -> p c n", p=P)
    wk_r = wk_d.rearrange("(c p) n -> p c n", p=P)
    wv_r = wv_d.rearrange("(c p) n -> p c n", p=P)
    wo_r = wo_d.rearrange("(c p) n -> p c n", p=P)
    bq_r = bq_d.rearrange("(c p) -> p c", p=P)
    bk_r = bk_d.rearrange("(c p) -> p c", p=P)
    bv_r = bv_d.rearrange("(a d) -> a d", a=1)

    EXP = mybir.ActivationFunctionType.Exp

    with tile.TileContext(nc) as tc, ExitStack() as ctx:
        const = ctx.enter_context(tc.tile_pool(name="const", bufs=1))
        big = ctx.enter_context(tc.tile_pool(name="big", bufs=1))
        wpool = ctx.enter_context(tc.tile_pool(name="wpool", bufs=4))
        work = ctx.enter_context(tc.tile_pool(name="work", bufs=3))
        probs_pool = ctx.enter_context(tc.tile_pool(name="probs", bufs=3))
        zsb = ctx.enter_context(tc.tile_pool(name="zsb", bufs=4))
        # PSUM budget (8 banks): scores 2x2 + z 2 + proj 2
        spsum = ctx.enter_context(tc.tile_pool(name="spsum", bufs=2, space="PSUM"))
        zpsum = ctx.enter_context(tc.tile_pool(name="zpsum", bufs=2, space="PSUM"))
        ppsum = ctx.enter_context(tc.tile_pool(name="ppsum", bufs=2, space="PSUM"))
        dramp = ctx.enter_context(tc.tile_pool(name="dramp", bufs=2, space="DRAM"))

        # ---- constants (on the ACT HWDGE ring, off the main input ring)
        bq_t = const.tile([P, NPAIR], F32)
        nc.scalar.dma_start(bq_t[:], bq_r[:, :])
        bk_t = const.tile([P, NPAIR], F32)
        nc.scalar.dma_start(bk_t[:], bk_r[:, :])
        bvb = const.tile([P, DQ], BF16)
        nc.scalar.dma_start(bvb[:], bv_r[0:1, :].to_broadcast((P, DQ)))

        # ---- X^T + weight loads. Ordered so the first attention stream
        # can start ~12us in: wk + first 512-token block of xt, then wq
        # (first scores), then wv (first PV), then the remaining xt
        # blocks (consumed progressively by K/V chains), wo last.
        xt_t = big.tile([P, DIN, S_full], BF16, tag="xt")
        wk_t = wpool.tile([P, DIN, DQ], BF16, tag="w", name="wk_t")
        wq_t = wpool.tile([P, DIN, DQ], BF16, tag="w", name="wq_t")
        wv_t = wpool.tile([P, DIN, DQ], BF16, tag="w", name="wv_t")
        wo_t = wpool.tile([P, DCO, D], BF16, tag="wo", name="wo_t")
        # weights go on the ACT HWDGE ring, xt/wk on the SP ring — the two
        # rings transfer in parallel, halving the serial input-DMA head
        for c in range(DIN):
            nc.sync.dma_start(wk_t[:, c, :], wk_r[:, c, :])
            nc.sync.dma_start(xt_t[:, c, 0:512], xt_r[:, c, 0:512])
        for c in range(DIN):
            nc.scalar.dma_start(wv_t[:, c, :], wv_r[:, c, :])
        for c in range(DIN):
            nc.scalar.dma_start(wq_t[:, c, :], wq_r[:, c, :])
        for tb in range(1, TOKC):
            for c in range(DIN):
                nc.sync.dma_start(xt_t[:, c, tb * 512:(tb + 1) * 512],
                                  xt_r[:, c, tb * 512:(tb + 1) * 512])
        for c in range(DCO):
            nc.scalar.dma_start(wo_t[:, c, :], wo_r[:, c, :])

        kt = [big.tile([P, S_full], BF16, tag=f"kt{p}", name=f"kt{p}")
              for p in range(NPAIR)]
        qt = [big.tile([P, S_full], BF16, tag=f"qt{p}", name=f"qt{p}")
              for p in range(NPAIR)]
        # per-pair augmented V: 2 heads x (64 data + 1 ones col)
        vt = [big.tile([P, NTOK, 130], BF16, tag=f"vt{p}", name=f"vt{p}")
              for p in range(NPAIR)]
        zt = [big.tile([P, S_full], BF16, tag=f"zt{p}", name=f"zt{p}")
              for p in range(NPAIR)]

        for pr in range(NPAIR):
            ones_ap = vt[pr].rearrange("p t (h c) -> p t h c", c=65)[:, :, :, 64:65]
            nc.vector.memset(ones_ap, 1.0)

        # ---- projection chain thunks (each thunk emits ~2 matmuls) ----
        def k_chain(pr, t):
            st = {}
            def mk(c0):
                def f():
                    if c0 == 0:
                        st['ps'] = ppsum.tile([P, 512], F32, tag="proj",
                                              name="psk")
                    for c in (c0, c0 + 1):
                        nc.tensor.matmul(
                            st['ps'][:],
                            wk_t[:, c, pr * P:(pr + 1) * P],
                            xt_t[:, c, t * 512:(t + 1) * 512],
                            start=(c == 0), stop=(c == DIN - 1),
                        )
                    if c0 == DIN - 2:
                        nc.vector.tensor_scalar_add(
                            kt[pr][:, t * 512:(t + 1) * 512], st['ps'][:],
                            bk_t[:, pr:pr + 1])
                return f
            return [mk(c0) for c0 in range(0, DIN, 2)]

        def q_chain(pr, t):
            st = {}
            def mk(c0):
                def f():
                    if c0 == 0:
                        st['ps'] = ppsum.tile([P, 512], F32, tag="proj",
                                              name="psq")
                    for c in (c0, c0 + 1):
                        nc.tensor.matmul(
                            st['ps'][:],
                            wq_t[:, c, pr * P:(pr + 1) * P],
                            xt_t[:, c, t * 512:(t + 1) * 512],
                            start=(c == 0), stop=(c == DIN - 1),
                        )
                    if c0 == DIN - 2:
                        nc.vector.tensor_scalar_add(
                            qt[pr][:, t * 512:(t + 1) * 512], st['ps'][:],
                            bq_t[:, pr:pr + 1])
                return f
            return [mk(c0) for c0 in range(0, DIN, 2)]

        def v_chain(g, t):
            prs = (2 * g, 2 * g + 1)
            st = {}
            def mk(c0):
                def f():
                    if c0 == 0:
                        st['ps'] = ppsum.tile([P, 256], F32, tag="proj",
                                              name="psv")
                    for c in (c0, c0 + 1):
                        nc.tensor.matmul(
                            st['ps'][:],
                            xt_t[:, c, t * P:(t + 1) * P],
                            wv_t[:, c, g * 256:(g + 1) * 256],
                            start=(c == 0), stop=(c == DIN - 1),
                        )
                    if c0 == DIN - 2:
                        for u in range(2):
                            dst = vt[prs[u]].rearrange(
                                "p t (h c) -> p t h c", c=65)[:, t, :, 0:64]
                            src = st['ps'][:, u * 128:(u + 1) * 128].rearrange(
                                "p (h c) -> p h c", c=64)
                            bsrc = bvb[:, (g * 256 + u * 128):
                                       (g * 256 + (u + 1) * 128)
                                       ].rearrange("p (h c) -> p h c", c=64)
                            nc.vector.tensor_add(dst, src, bsrc)
                return f
            return [mk(c0) for c0 in range(0, DIN, 2)]

        def o_chain(qc, ec):
            oqsl = slice(qc * 512, (qc + 1) * 512)
            st = {}
            def mk(d0):
                def f():
                    if d0 == 0:
                        st['ps'] = ppsum.tile([P, 512], F32, tag="proj",
                                              name="pso")
                    for dc in (d0, d0 + 1):
                        nc.tensor.matmul(
                            st['ps'][:],
                            wo_t[:, dc, ec * P:(ec + 1) * P],
                            zt[dc][:, oqsl],
                            start=(dc == 0), stop=(dc == DCO - 1),
                        )
                    if d0 == DCO - 2:
                        stg = work.tile([P, 512], BF16, tag="stage",
                                        name="st", bufs=2)
                        nc.vector.tensor_copy(stg[:], st['ps'][:])
                        nc.sync.dma_start(
                            ot_d[ec * P:(ec + 1) * P, oqsl], stg[:])
                return f
            return [mk(d0) for d0 in range(0, DCO, 2)]

        # ---- filler queue: (deadline_stream, thunk), emitted in order ----
        fillers = []

        def push(deadline, thunks):
            for th in thunks:
                fillers.append((deadline, th))

        # head work (before stream 0): K(0,t0), Q(0,qc0), V(quad0) t0/t1;
        # the rest of K(0)/V(quad0) is emitted inside stream 0 (V chains
        # just-in-time, one per kc, two iterations ahead of the PV that
        # consumes them).
        # emit in DMA-arrival order (wk/xt, wv, wq) so the in-order PE
        # queue never head-blocks on a later-arriving weight
        for th in k_chain(0, 0):
            th()
        for th in v_chain(0, 0):
            th()
        for th in v_chain(0, 1):
            th()
        for th in q_chain(0, 0):
            th()

        # stream s = pr*4 + qc; deadline = stream during which to emit.
        # Every producer's deadline is < its first consumer stream; loads
        # are spread so budget-2 pops drain each stream's quota in-stream.
        for t in range(1, TOKC):
            push(0, k_chain(0, t))
        push(0, q_chain(0, 1))
        push(1, k_chain(1, 0)); push(1, k_chain(1, 1))
        push(1, q_chain(0, 2))
        push(2, k_chain(1, 2)); push(2, k_chain(1, 3))
        push(2, q_chain(0, 3))
        push(3, q_chain(1, 0))
        for t in range(NTOK):
            push(4 + t // 4, v_chain(1, t))
        push(4, q_chain(1, 1))
        push(5, q_chain(1, 2))
        push(6, k_chain(2, 0)); push(6, k_chain(2, 1))
        push(6, q_chain(1, 3))
        push(7, k_chain(2, 2)); push(7, k_chain(2, 3))
        push(7, q_chain(2, 0))
        push(8, q_chain(2, 1))
        push(9, q_chain(2, 2))
        push(10, k_chain(3, 0)); push(10, k_chain(3, 1))
        push(10, q_chain(2, 3))
        push(11, k_chain(3, 2)); push(11, k_chain(3, 3))
        push(11, q_chain(3, 0))
        push(12, q_chain(3, 1))
        push(13, [th for ec in range(DIN) for th in o_chain(0, ec)])
        push(13, q_chain(3, 2))
        push(14, [th for ec in range(DIN) for th in o_chain(1, ec)])
        push(14, q_chain(3, 3))
        push(15, [th for ec in range(DIN) for th in o_chain(2, ec)])
        push(16, [th for ec in range(DIN) for th in o_chain(3, ec)])

        fillers.sort(key=lambda x: x[0])
        fillers = list(reversed(fillers))  # pop from end

        def pop_fillers(s, budget):
            n = 0
            while fillers and (fillers[-1][0] < s
                               or (fillers[-1][0] <= s and n < budget)):
                fillers.pop()[1]()
                n += 1

        # ---- one attention stream: (pair pr, 512-query chunk qc) ----
        def attn(pr, qc, s):
            # Emission order IS program semantics (Tile serializes a
            # later-emitted write AFTER an earlier-emitted read of the same
            # region), so every producer chain with deadline < s must be
            # fully emitted before this stream's first scores read.
            pop_fillers(s, 0)
            vpr = vt[pr].rearrange("p t (h c) -> p t h c", c=65)
            qsl = slice(qc * 512, (qc + 1) * 512)
            za = zpsum.tile([P, 512], F32, tag="z", name=f"za{pr}_{qc}")
            zb = zpsum.tile([P, 512], F32, tag="z", name=f"zb{pr}_{qc}")

            def emit_scores(kc):
                sq = spsum.tile([P, 2, 512], F32, tag="sc", name="sq")
                nc.tensor.matmul(
                    sq[:, 0, :],
                    kt[pr][0:64, kc * P:(kc + 1) * P],
                    qt[pr][0:64, qsl],
                    start=True, stop=True,
                )
                nc.tensor.matmul(
                    sq[:, 1, :],
                    kt[pr][64:128, kc * P:(kc + 1) * P],
                    qt[pr][64:128, qsl],
                    start=True, stop=True,
                )
                return sq

            def emit_act(sq):
                pq = probs_pool.tile([P, 2, 512], BF16, tag="probs",
                                     name="pq")
                nc.scalar.activation(pq[:], sq[:], EXP, scale=0.125)
                return pq

            def emit_pv(kc, pq):
                nc.tensor.matmul(
                    za[0:65, :], vpr[:, kc, 0, :], pq[:, 0, :],
                    start=(kc == 0), stop=(kc == KC - 1),
                )
                nc.tensor.matmul(
                    zb[0:65, :], vpr[:, kc, 1, :], pq[:, 1, :],
                    start=(kc == 0), stop=(kc == KC - 1),
                )

            # software pipeline: ACT(kc+1), scores(kc+2), fillers, PV(kc)
            sq0 = emit_scores(0)
            pq_cur = emit_act(sq0)
            sq_next = emit_scores(1)
            for kc in range(KC):
                pq_next = None
                if kc + 1 < KC:
                    pq_next = emit_act(sq_next)
                if kc + 2 < KC:
                    sq_next = emit_scores(kc + 2)
                pop_fillers(s, 2)
                emit_pv(kc, pq_cur)
                if s == 0 and kc + 2 < KC:
                    # just-in-time V(quad0) chain for kc+2, two iterations
                    # ahead of the PV that reads it
                    for th in v_chain(0, kc + 2):
                        th()
                pq_cur = pq_next

            # evict z chains to SBUF (frees the z banks), then normalize
            zaf = zsb.tile([P, 512], F32, tag="zf", name="zaf")
            nc.vector.tensor_copy(zaf[0:65, :], za[0:65, :])
            zbf = zsb.tile([P, 512], F32, tag="zf", name="zbf")
            nc.vector.tensor_copy(zbf[0:65, :], zb[0:65, :])
            rsc = dramp.tile([2, 512], F32, tag="rsc", name="rsc")
            nc.sync.dma_start(rsc[0:1, :], zaf[64:65, :])
            nc.sync.dma_start(rsc[1:2, :], zbf[64:65, :])
            rbr = work.tile([P, 2, 512], F32, tag="rbc", name="rbr", bufs=1)
            nc.sync.dma_start(rbr[0:64, 0, :],
                              rsc[0:1, :].to_broadcast((64, 512)))
            nc.sync.dma_start(rbr[0:64, 1, :],
                              rsc[1:2, :].to_broadcast((64, 512)))
            rb = work.tile([P, 2, 512], F32, tag="rbc2", name="rb", bufs=1)
            nc.vector.reciprocal_approx_fast(rb[0:64, :, :],
                                             rbr[0:64, :, :])
            nc.vector.tensor_mul(zt[pr][0:64, qsl], zaf[0:64, :],
                                 rb[0:64, 0, :])
            zs = work.tile([P, 512], BF16, tag="zstage", name="zs", bufs=2)
            nc.vector.tensor_mul(zs[0:64, :], zbf[0:64, :],
                                 rb[0:64, 1, :])
            nc.sync.dma_start(zt[pr][64:128, qsl], zs[0:64, :])

        for pr in range(NPAIR):
            for qc in range(QCN):
                attn(pr, qc, pr * QCN + qc)

        # drain remaining fillers (O projections for the last chunks)
        while fillers:
            fillers.pop()[1]()

    nc.compile()
    return nc



# ---------------- host-side entry point ----------------

BF = ml_dtypes.bfloat16
_B, _S, _D, _H = 4, 2048, 1024, 16
_DQ = _D // 2
_NC_CACHE = None


def _get_nc():
    global _NC_CACHE
    if _NC_CACHE is None:
        _NC_CACHE = build_attention_nc(S_full=_S, D=_D, DQ=_DQ)
    return _NC_CACHE


def kernel(X, Wq, bq, Wk, bk, Wv, bv, Wo, bo):
    """Full-input multi-head attention on 8 TRN2 NeuronCores.

    Sharding: core c handles batch c//2 and head-half c%2 (8 of 16
    heads, column shards of Wq/Wk/Wv, row shard of Wo). Each core
    returns a partial output projection; the host sums the two
    partials per batch and adds bo. Inputs are cast to bf16 on host.
    """
    from concourse.bass_utils import run_bass_kernel_spmd

    X = np.asarray(X, dtype=np.float32)
    bq = np.asarray(bq, dtype=np.float32)
    bk = np.asarray(bk, dtype=np.float32)
    bvb = np.ascontiguousarray(np.asarray(bv, dtype=np.float32).astype(BF))
    bo = np.asarray(bo, dtype=np.float32)
    wqb = np.asarray(Wq, dtype=np.float32).astype(BF)
    wkb = np.asarray(Wk, dtype=np.float32).astype(BF)
    wvb = np.asarray(Wv, dtype=np.float32).astype(BF)
    wob = np.asarray(Wo, dtype=np.float32).astype(BF)

    in_maps = []
    xtb = [np.ascontiguousarray(X[b].T.astype(BF)) for b in range(_B)]
    for c in range(8):
        b, half = c // 2, c % 2
        hsl = slice(half * _DQ, (half + 1) * _DQ)
        in_maps.append({
            "xt": xtb[b],
            "wq": np.ascontiguousarray(wqb[:, hsl]),
            "wk": np.ascontiguousarray(wkb[:, hsl]),
            "wv": np.ascontiguousarray(wvb[:, hsl]),
            "wo": np.ascontiguousarray(wob[hsl, :]),
            "bq": np.ascontiguousarray(bq[hsl]),
            "bk": np.ascontiguousarray(bk[hsl]),
            "bv": np.ascontiguousarray(bvb[hsl]),
        })

    nc = _get_nc()
    res = run_bass_kernel_spmd(nc, in_maps, list(range(8)))

    out = np.empty((_B, _S, _D), np.float32)
    for b in range(_B):
        acc = (res.results[2 * b]["ot"].astype(np.float32)
               + res.results[2 * b + 1]["ot"].astype(np.float32))
        out[b] = acc.T + bo
    return out
